# revision 10
# baseline (speedup 1.0000x reference)
"""GNN neighborhood aggregation (gather + mean) on 8 TRN2 NeuronCores.

out = features[concat([nodes[:,None], neigh_idx],1)].mean(1)  # [50k,128]

Data-parallel over seeds (6250/core), feature table replicated (fp16).
Per core the 33*6250 row refs are sorted by (seed-half, bank-of-32768,
seed, row); one gather stream per (half, bank), chunked to <=13*128
descriptors (fits the SWDGE descriptor ring, so Q7 desc-gen never
blocks the GpSimd engine) and issued round-robin across 4 SWDGE queues
so the 16 DMA engines keep 4 outstanding descriptor rings each.
Measured ~2.1 ns/descriptor aggregate vs 8.4 on one queue (v1
serialized all gathers on queue 0: 1.99 ms GpSimd busy).

The reduce runs entirely in PSUM: out[f, seed] += matmul(
lhsT=G_slot[pos, feat] fp16, rhs=SEL[pos, seed_cols] fp8e4,
start=False) onto a DVE-zeroed psum region holding half the seed
tiles (25/24 tiles x 128 seeds). One matmul per contiguous span piece
per 128-pos slot (split only at 512-col psum bank boundaries). SELs
are host-built one-hots (1.0 is exact in e4m3), column-packed per slot
span and streamed as fp8 blocks on the otherwise-idle HWDGE queue
(45 MB/core; bf16 was 90). No per-entry DVE adds (v1: 4816
tensor_tensor = 1.65 ms) and no per-entry accumulator traffic.
Final: one scale(1/33) per half (ACT / DVE) -> out dram [128, 6272]
f32, transposed on host. Schedule is data-dependent, compiled on
first call, cached by input fingerprint.

Measured: 511934 ns HW exec (baseline 2131560), rel err 2.1e-4.
"""

import sys

if "/opt/trn_rl_repo" not in sys.path:
    sys.path.insert(0, "/opt/trn_rl_repo")

import numpy as np
import ml_dtypes

N_NODES = 1_000_000
D = 128
B = 50_000
K = 33
NCORES = 8
B_LOC = B // NCORES
P = 128
NT = (B_LOC + P - 1) // P  # 49 tiles of seeds
T_SPLIT = 25  # tiles 0..24 in half 0, 25..48 in half 1
BANK = 32768
NQ = 4  # SWDGE queues
G_BUFS = 8
CHUNK_SLOTS = 13  # gather chunk size: 13*128=1664 descs fits the desc ring
SEL_BLOCK = 4096  # fp8 sel cols per DMA block (4KB/partition lines)
BANKW = 512  # psum bank width in f32 cols
SEL_W = 512  # sel tile width (>= max span piece... spans can exceed; split)


def wrap16(s):
    n = len(s)
    assert n % 16 == 0
    a = np.asarray(s, dtype=np.int16).reshape(n // 16, 16).T
    return np.tile(a, (8, 1))


def prep3(nodes, neigh_idx, n_nodes=N_NODES, bank=BANK, ncores=NCORES,
          t_split=T_SPLIT, k=K):
    b_loc = nodes.shape[0] // ncores
    nt = (b_loc + P - 1) // P
    idx_all = np.concatenate(
        [np.asarray(nodes)[:, None], np.asarray(neigh_idx)], axis=1
    ).astype(np.int64)  # [B, K]
    nbanks = (n_nodes + bank - 1) // bank
    seed_split = t_split * P

    per = {}
    for c in range(ncores):
        rows = idx_all[c * b_loc : (c + 1) * b_loc].reshape(-1)
        seeds = np.repeat(np.arange(b_loc, dtype=np.int64), k)
        bid = rows // bank
        half = (seeds >= seed_split).astype(np.int64)
        order = np.lexsort((rows, seeds, bid, half))
        rows, seeds, bid, half = rows[order], seeds[order], bid[order], half[order]
        for h in (0, 1):
            for bb in range(nbanks):
                m = (half == h) & (bid == bb)
                per[c, h, bb] = (rows[m] - bb * bank, seeds[m])

    # streams in program order: (half, bank, padded len)
    streams = []
    for h in (0, 1):
        for bb in range(nbanks):
            mx = max(len(per[c, h, bb][0]) for c in range(ncores))
            L = ((mx + P - 1) // P) * P
            if L:
                streams.append((h, bb, L))

    tot = sum(L for _, _, L in streams)
    gidx = np.zeros((ncores, tot), np.int64)
    keys = np.full((ncores, tot), -1, np.int64)
    off = 0
    stream_off = []
    for si, (h, bb, L) in enumerate(streams):
        stream_off.append(off)
        for c in range(ncores):
            r, s = per[c, h, bb]
            gidx[c, off : off + len(r)] = r
            keys[c, off : off + len(s)] = s
        off += L

    # per-slot span pieces + packed one-hot SEL columns.
    # slot_recs: (stream_idx, slot, lo, span, soff, [(pc0, w), ...])
    nslots_tot = tot // P
    slot_recs = []
    blocks = []  # (first_stream_idx, sel col start, width)
    sel_pos = []  # (core array fill info) collected then applied
    soff = 0
    blk_start, blk_w, blk_si = 0, 0, 0
    for si, (h, bb, L) in enumerate(streams):
        so = stream_off[si]
        h_base = 0 if h == 0 else t_split * P
        for sl in range(L // P):
            kk = keys[:, so + sl * P : so + (sl + 1) * P]  # [ncores, 128]
            valid = kk >= 0
            lc = np.where(valid, kk - h_base, -1)
            if valid.any():
                lo = int(lc[valid].min())
                hi = int(lc[valid].max())
                span = hi + 1 - lo
                if blk_w + span > SEL_BLOCK and blk_w > 0:
                    blocks.append((blk_si, blk_start, blk_w))
                    blk_start, blk_w = soff, 0
                if blk_w == 0:
                    blk_si = si
                pieces = []
                for bkb in range(lo // BANKW, hi // BANKW + 1):
                    pc0 = max(lo, bkb * BANKW)
                    pc1 = min(hi + 1, (bkb + 1) * BANKW)
                    pieces.append((pc0, pc1 - pc0))
                slot_recs.append((si, sl, lo, span, soff, pieces))
                for c in range(ncores):
                    q = np.nonzero(valid[c])[0]
                    sel_pos.append((c, q, soff + (lc[c, q] - lo)))
                soff += span
                blk_w += span
        # flush block at stream end so program emission stays stream-local
        if blk_w > 0:
            blocks.append((blk_si, blk_start, blk_w))
            blk_start, blk_w = soff, 0
    wtot = ((soff + 15) // 16) * 16
    sels = np.zeros((ncores, P, wtot), np.uint8)
    one_u8 = np.array(1.0, dtype=ml_dtypes.float8_e4m3).view(np.uint8)
    for c, q, cols in sel_pos:
        sels[c, q, cols] = one_u8

    gidx_cat = np.stack([
        np.concatenate(
            [wrap16(gidx[c, stream_off[si] : stream_off[si] + L])
             for si, (_, _, L) in enumerate(streams)], axis=1)
        for c in range(ncores)
    ])  # [ncores, 128, tot/16]

    meta = dict(
        streams=[(h, bb, L) for h, bb, L in streams],
        slot_recs=slot_recs,
        blocks=blocks,
        nt=nt,
        t_split=t_split,
        tot=tot,
        nslots_tot=nslots_tot,
        wtot=wtot,
    )
    return gidx_cat, sels, meta


def build3(meta, n_nodes=N_NODES, bank=BANK, nq=NQ, g_bufs=G_BUFS, sel_bufs=8):
    import concourse.bacc as bacc
    import concourse.tile as tile
    from concourse import mybir

    streams = meta["streams"]
    slot_recs = meta["slot_recs"]
    nt, t_split = meta["nt"], meta["t_split"]
    tot, nslots_tot = meta["tot"], meta["nslots_tot"]
    ncols = [t_split * P, (nt - t_split) * P]
    nbank_h = [(c * 4 + 2047) // 2048 for c in ncols]
    iota_w = max(nbank_h) * BANKW

    nc = bacc.Bacc(
        "TRN2",
        target_bir_lowering=False,
        debug=False,
        num_devices=NCORES,
        num_swdge_queues=nq,
    )
    feat = nc.dram_tensor(
        "features_f16", [n_nodes, D], mybir.dt.float16, kind="ExternalInput"
    ).ap()
    gidx = nc.dram_tensor(
        "gidx", [P, tot // 16], mybir.dt.int16, kind="ExternalInput"
    ).ap()
    sels = nc.dram_tensor(
        "sels", [P, meta["wtot"]], mybir.dt.float8e4, kind="ExternalInput"
    ).ap()
    out = nc.dram_tensor(
        "out", [P, nt * P], mybir.dt.float32, kind="ExternalOutput"
    ).ap()

    blocks = meta["blocks"]
    blocks_of = [[] for _ in streams]
    for bi, (bsi, bstart, bw) in enumerate(blocks):
        blocks_of[bsi].append((bi, bstart, bw))
    # slots grouped per block (slots and blocks are both in sel-col order)
    recs_of_block = [[] for _ in blocks]
    ri = 0
    for bi, (bsi, bstart, bw) in enumerate(blocks):
        while ri < len(slot_recs) and slot_recs[ri][4] < bstart + bw:
            assert slot_recs[ri][4] >= bstart
            recs_of_block[bi].append(slot_recs[ri])
            ri += 1
    assert ri == len(slot_recs)

    with tile.TileContext(nc) as tc:
        with tc.tile_pool(name="fix", bufs=1) as fx, tc.tile_pool(
            name="g", bufs=g_bufs
        ) as gp, tc.tile_pool(name="sel", bufs=sel_bufs) as sp, tc.tile_pool(
            name="ps", bufs=1, space="PSUM"
        ) as pp, tc.tile_pool(name="o", bufs=2) as op:
            idx_t = fx.tile([P, tot // 16], mybir.dt.int16, tag="idx")
            nc.sync.dma_start(out=idx_t[:], in_=gidx[:])

            off16 = 0
            gslot = 0
            qctr = [0]
            for h in (0, 1):
                ps = pp.tile([P, nbank_h[h] * BANKW], mybir.dt.float32, tag="ps")
                nc.vector.memset(ps[:], 0.0)
                h_streams = [s for s in range(len(streams)) if streams[s][0] == h]
                for s in h_streams:
                    hh, bb, L = streams[s]
                    base = bb * bank
                    rows = min(bank, n_nodes - base)
                    nslots = L // P
                    G = gp.tile([P, nslots * D], mybir.dt.float16, tag="G")
                    nchunk = (nslots + CHUNK_SLOTS - 1) // CHUNK_SLOTS
                    bounds = [(nslots * j) // nchunk for j in range(nchunk + 1)]
                    for j in range(nchunk):
                        s0, s1 = bounds[j], bounds[j + 1]
                        ln = (s1 - s0) * P
                        nc.gpsimd.dma_gather(
                            out_ap=G[:, s0 * D : s1 * D].rearrange(
                                "p (s d) -> p s d", d=D
                            ),
                            in_ap=feat[base : base + rows, :],
                            idxs_ap=idx_t[
                                :, off16 + s0 * 8 : off16 + s1 * 8
                            ],
                            num_idxs=ln,
                            num_idxs_reg=ln,
                            elem_size=D,
                            single_packet=False,
                            queue_num=qctr[0] % nq,
                        )
                        qctr[0] += 1
                    off16 += L // 16
                    for (bi, bstart, bw) in blocks_of[s]:
                        st = sp.tile([P, SEL_BLOCK], mybir.dt.float8e4,
                                     tag="sel")
                        nc.sync.dma_start(
                            out=st[:, :bw], in_=sels[:, bstart : bstart + bw]
                        )
                        for (_si, sl, lo, span, soff, pieces) in \
                                recs_of_block[bi]:
                            assert _si == s
                            for (pc0, w) in pieces:
                                o = soff - bstart + pc0 - lo
                                nc.tensor.matmul(
                                    out=ps[:, pc0 : pc0 + w],
                                    lhsT=G[:, sl * D : (sl + 1) * D],
                                    rhs=st[:, o : o + w],
                                    start=False,
                                    stop=True,
                                    skip_group_check=True,
                                )
                ot = op.tile([P, ncols[h]], mybir.dt.float32, tag="ot")
                if h == 0:
                    nc.scalar.mul(ot[:], ps[:, : ncols[h]], 1.0 / K)
                else:
                    nc.vector.tensor_scalar_mul(ot[:], ps[:, : ncols[h]], 1.0 / K)
                ocol = 0 if h == 0 else t_split * P
                nc.sync.dma_start(
                    out=out[:, ocol : ocol + ncols[h]], in_=ot[:]
                )
    nc.compile()
    return nc


PROFILE = False
_cache = {"key": None, "nc": None, "meta": None}


def kernel(features, nodes, neigh_idx):
    from concourse import bass_utils

    features = np.ascontiguousarray(np.asarray(features), dtype=np.float32)
    nodes = np.asarray(nodes)
    neigh_idx = np.asarray(neigh_idx)
    key = (nodes.tobytes(), neigh_idx.tobytes())
    if _cache["key"] != key:
        gidx_cat, sels, meta = prep3(nodes, neigh_idx)
        nc = build3(meta)
        _cache.update(key=key, nc=nc, meta=(gidx_cat, sels, meta))
    nc = _cache["nc"]
    gidx_cat, sels, meta = _cache["meta"]
    feat_f16 = features.astype(np.float16)
    in_maps = [
        {
            "features_f16": feat_f16,
            "gidx": np.ascontiguousarray(gidx_cat[c]),
            "sels": np.ascontiguousarray(
                sels[c].view(ml_dtypes.float8_e4m3)
            ),
        }
        for c in range(NCORES)
    ]
    res = bass_utils.run_bass_kernel_spmd(
        nc,
        in_maps,
        core_ids=list(range(NCORES)),
        trace=PROFILE,
        trace_cores=[0] if PROFILE else None,
    )
    if PROFILE:
        kernel.last_result = res
    out = np.concatenate(
        [res.results[c]["out"].T[:B_LOC] for c in range(NCORES)], axis=0
    )
    return np.ascontiguousarray(out, dtype=np.float32)


# revision 14
# speedup vs baseline: 1.1847x; 1.1847x over previous
"""GNN neighborhood aggregation (gather + mean) on 8 TRN2 NeuronCores.

out = features[concat([nodes[:,None], neigh_idx],1)].mean(1)  # [50k,128]

Data-parallel over seeds (6250/core), feature table replicated (fp16).
Per core the 33*6250 row refs are sorted by (seed-half, bank-of-32768,
seed, row); one gather stream per (half, bank), chunked to <=13*128
descriptors (fits the SWDGE descriptor ring, so Q7 desc-gen never
blocks the GpSimd engine) and issued round-robin across 4 SWDGE queues
so the 16 DMA engines keep 4 outstanding descriptor rings each.
Measured ~2.1 ns/descriptor aggregate vs 8.4 on one queue (v1
serialized all gathers on queue 0: 1.99 ms GpSimd busy).

The reduce runs entirely in PSUM: out[f, seed] += matmul(
lhsT=G_slot[pos, feat] fp16, rhs=SEL[pos, seed_cols] fp8e4,
start=False) onto a DVE-zeroed psum region holding half the seed
tiles (25/24 tiles x 128 seeds). One matmul per contiguous span piece
per 128-pos slot (split only at 512-col psum bank boundaries). SELs
are host-built one-hots (1.0 is exact in e4m3), column-packed per slot
span and streamed as fp8 blocks on the otherwise-idle HWDGE queue
(45 MB/core; bf16 was 90). No per-entry DVE adds (v1: 4816
tensor_tensor = 1.65 ms) and no per-entry accumulator traffic.
Final: one scale(1/33) per half (ACT / DVE) -> out dram [128, 6272]
f32, transposed on host. Schedule is data-dependent, compiled on
first call, cached by input fingerprint.

Measured: 511934 ns HW exec (baseline 2131560), rel err 2.1e-4.
"""

import sys

if "/opt/trn_rl_repo" not in sys.path:
    sys.path.insert(0, "/opt/trn_rl_repo")

import numpy as np
import ml_dtypes

N_NODES = 1_000_000
D = 128
B = 50_000
K = 33
NCORES = 8
B_LOC = B // NCORES
P = 128
NT = (B_LOC + P - 1) // P  # 49 tiles of seeds
T_SPLIT = 25  # tiles 0..24 in half 0, 25..48 in half 1
BANK = 32768
NQ = 4  # SWDGE queues
G_BUFS = 8
CHUNK_SLOTS = 13  # gather chunk size: 13*128=1664 descs fits the desc ring
SEL_BLOCK = 4096  # fp8 sel cols per DMA block (4KB/partition lines)
BANKW = 512  # psum bank width in f32 cols
SEL_W = 512  # sel tile width (>= max span piece... spans can exceed; split)


def wrap16(s):
    n = len(s)
    assert n % 16 == 0
    a = np.asarray(s, dtype=np.int16).reshape(n // 16, 16).T
    return np.tile(a, (8, 1))


def prep3(nodes, neigh_idx, n_nodes=N_NODES, bank=BANK, ncores=NCORES,
          t_split=T_SPLIT, k=K):
    b_loc = nodes.shape[0] // ncores
    nt = (b_loc + P - 1) // P
    idx_all = np.concatenate(
        [np.asarray(nodes)[:, None], np.asarray(neigh_idx)], axis=1
    ).astype(np.int64)  # [B, K]
    nbanks = (n_nodes + bank - 1) // bank
    seed_split = t_split * P

    per = {}
    for c in range(ncores):
        rows = idx_all[c * b_loc : (c + 1) * b_loc].reshape(-1)
        seeds = np.repeat(np.arange(b_loc, dtype=np.int64), k)
        bid = rows // bank
        half = (seeds >= seed_split).astype(np.int64)
        order = np.lexsort((rows, seeds, bid, half))
        rows, seeds, bid, half = rows[order], seeds[order], bid[order], half[order]
        for h in (0, 1):
            for bb in range(nbanks):
                m = (half == h) & (bid == bb)
                per[c, h, bb] = (rows[m] - bb * bank, seeds[m])

    # streams in program order: (half, bank, padded len)
    streams = []
    for h in (0, 1):
        for bb in range(nbanks):
            mx = max(len(per[c, h, bb][0]) for c in range(ncores))
            L = ((mx + P - 1) // P) * P
            if L:
                streams.append((h, bb, L))

    tot = sum(L for _, _, L in streams)
    gidx = np.zeros((ncores, tot), np.int64)
    keys = np.full((ncores, tot), -1, np.int64)
    off = 0
    stream_off = []
    for si, (h, bb, L) in enumerate(streams):
        stream_off.append(off)
        for c in range(ncores):
            r, s = per[c, h, bb]
            gidx[c, off : off + len(r)] = r
            keys[c, off : off + len(s)] = s
        off += L

    # per-slot span pieces + packed one-hot SEL columns.
    # slot_recs: (stream_idx, slot, lo, span, soff, [(pc0, w), ...])
    nslots_tot = tot // P
    slot_recs = []
    blocks = []  # (first_stream_idx, sel col start, width)
    sel_pos = []  # (core array fill info) collected then applied
    soff = 0
    blk_start, blk_w, blk_si = 0, 0, 0
    for si, (h, bb, L) in enumerate(streams):
        so = stream_off[si]
        h_base = 0 if h == 0 else t_split * P
        for sl in range(L // P):
            kk = keys[:, so + sl * P : so + (sl + 1) * P]  # [ncores, 128]
            valid = kk >= 0
            lc = np.where(valid, kk - h_base, -1)
            if valid.any():
                lo = int(lc[valid].min())
                hi = int(lc[valid].max())
                span = hi + 1 - lo
                if blk_w + span > SEL_BLOCK and blk_w > 0:
                    blocks.append((blk_si, blk_start, blk_w))
                    blk_start, blk_w = soff, 0
                if blk_w == 0:
                    blk_si = si
                pieces = []
                for bkb in range(lo // BANKW, hi // BANKW + 1):
                    pc0 = max(lo, bkb * BANKW)
                    pc1 = min(hi + 1, (bkb + 1) * BANKW)
                    pieces.append((pc0, pc1 - pc0))
                slot_recs.append((si, sl, lo, span, soff, pieces))
                for c in range(ncores):
                    q = np.nonzero(valid[c])[0]
                    sel_pos.append((c, q, soff + (lc[c, q] - lo)))
                soff += span
                blk_w += span
        # flush block at stream end so program emission stays stream-local
        if blk_w > 0:
            blocks.append((blk_si, blk_start, blk_w))
            blk_start, blk_w = soff, 0
    wtot = ((soff + 15) // 16) * 16
    sels = np.zeros((ncores, P, wtot), np.uint8)
    one_u8 = np.array(1.0, dtype=ml_dtypes.float8_e4m3).view(np.uint8)
    for c, q, cols in sel_pos:
        sels[c, q, cols] = one_u8

    gidx_cat = np.stack([
        np.concatenate(
            [wrap16(gidx[c, stream_off[si] : stream_off[si] + L])
             for si, (_, _, L) in enumerate(streams)], axis=1)
        for c in range(ncores)
    ])  # [ncores, 128, tot/16]

    meta = dict(
        streams=[(h, bb, L) for h, bb, L in streams],
        slot_recs=slot_recs,
        blocks=blocks,
        nt=nt,
        t_split=t_split,
        tot=tot,
        nslots_tot=nslots_tot,
        wtot=wtot,
    )
    return gidx_cat, sels, meta


def build3(meta, n_nodes=N_NODES, bank=BANK, nq=NQ, g_bufs=G_BUFS, sel_bufs=8):
    import concourse.bacc as bacc
    import concourse.tile as tile
    from concourse import mybir

    streams = meta["streams"]
    slot_recs = meta["slot_recs"]
    nt, t_split = meta["nt"], meta["t_split"]
    tot, nslots_tot = meta["tot"], meta["nslots_tot"]
    ncols = [t_split * P, (nt - t_split) * P]
    nbank_h = [(c * 4 + 2047) // 2048 for c in ncols]
    iota_w = max(nbank_h) * BANKW

    nc = bacc.Bacc(
        "TRN2",
        target_bir_lowering=False,
        debug=False,
        num_devices=NCORES,
        num_swdge_queues=nq,
    )
    feat = nc.dram_tensor(
        "features_f16", [n_nodes, D], mybir.dt.float16, kind="ExternalInput"
    ).ap()
    gidx = nc.dram_tensor(
        "gidx", [P, tot // 16], mybir.dt.int16, kind="ExternalInput"
    ).ap()
    sels = nc.dram_tensor(
        "sels", [P, meta["wtot"]], mybir.dt.float8e4, kind="ExternalInput"
    ).ap()
    out = nc.dram_tensor(
        "out", [P, nt * P], mybir.dt.float32, kind="ExternalOutput"
    ).ap()

    blocks = meta["blocks"]
    blocks_of = [[] for _ in streams]
    for bi, (bsi, bstart, bw) in enumerate(blocks):
        blocks_of[bsi].append((bi, bstart, bw))
    # slots grouped per block (slots and blocks are both in sel-col order)
    recs_of_block = [[] for _ in blocks]
    ri = 0
    for bi, (bsi, bstart, bw) in enumerate(blocks):
        while ri < len(slot_recs) and slot_recs[ri][4] < bstart + bw:
            assert slot_recs[ri][4] >= bstart
            recs_of_block[bi].append(slot_recs[ri])
            ri += 1
    assert ri == len(slot_recs)

    with tile.TileContext(nc) as tc:
        with tc.tile_pool(name="fix", bufs=1) as fx, tc.tile_pool(
            name="g", bufs=g_bufs
        ) as gp, tc.tile_pool(name="sel", bufs=sel_bufs) as sp, tc.tile_pool(
            name="ps", bufs=1, space="PSUM"
        ) as pp, tc.tile_pool(name="o", bufs=2) as op:
            idx_t = fx.tile([P, tot // 16], mybir.dt.int16, tag="idx")
            nc.sync.dma_start(out=idx_t[:], in_=gidx[:])

            off16 = 0
            gslot = 0
            qctr = [0]
            for h in (0, 1):
                ps = pp.tile([P, nbank_h[h] * BANKW], mybir.dt.float32, tag="ps")
                nc.vector.memset(ps[:], 0.0)
                h_streams = [s for s in range(len(streams)) if streams[s][0] == h]
                for s in h_streams:
                    hh, bb, L = streams[s]
                    base = bb * bank
                    rows = min(bank, n_nodes - base)
                    nslots = L // P
                    G = gp.tile([P, nslots * D], mybir.dt.float16, tag="G")
                    nchunk = (nslots + CHUNK_SLOTS - 1) // CHUNK_SLOTS
                    bounds = [(nslots * j) // nchunk for j in range(nchunk + 1)]
                    for j in range(nchunk):
                        s0, s1 = bounds[j], bounds[j + 1]
                        ln = (s1 - s0) * P
                        nc.gpsimd.dma_gather(
                            out_ap=G[:, s0 * D : s1 * D].rearrange(
                                "p (s d) -> p s d", d=D
                            ),
                            in_ap=feat[base : base + rows, :],
                            idxs_ap=idx_t[
                                :, off16 + s0 * 8 : off16 + s1 * 8
                            ],
                            num_idxs=ln,
                            num_idxs_reg=ln,
                            elem_size=D,
                            single_packet=False,
                            queue_num=qctr[0] % nq,
                        )
                        qctr[0] += 1
                    off16 += L // 16
                    for (bi, bstart, bw) in blocks_of[s]:
                        st = sp.tile([P, SEL_BLOCK], mybir.dt.float8e4,
                                     tag="sel")
                        nc.sync.dma_start(
                            out=st[:, :bw], in_=sels[:, bstart : bstart + bw]
                        )
                        for (_si, sl, lo, span, soff, pieces) in \
                                recs_of_block[bi]:
                            assert _si == s
                            for (pc0, w) in pieces:
                                o = soff - bstart + pc0 - lo
                                nc.tensor.matmul(
                                    out=ps[:, pc0 : pc0 + w],
                                    lhsT=G[:, sl * D : (sl + 1) * D],
                                    rhs=st[:, o : o + w],
                                    start=False,
                                    stop=True,
                                    skip_group_check=True,
                                )
                ot = op.tile([P, ncols[h]], mybir.dt.float32, tag="ot")
                if h == 0:
                    nc.scalar.mul(ot[:], ps[:, : ncols[h]], 1.0 / K)
                else:
                    nc.vector.tensor_scalar_mul(ot[:], ps[:, : ncols[h]], 1.0 / K)
                ocol = 0 if h == 0 else t_split * P
                nc.sync.dma_start(
                    out=out[:, ocol : ocol + ncols[h]], in_=ot[:]
                )
    nc.compile()
    return nc


PROFILE = False
_cache = {"key": None, "nc": None, "meta": None}


def kernel(features, nodes, neigh_idx):
    from concourse import bass_utils

    features = np.ascontiguousarray(np.asarray(features), dtype=np.float32)
    nodes = np.asarray(nodes)
    neigh_idx = np.asarray(neigh_idx)
    key = (nodes.tobytes(), neigh_idx.tobytes())
    if _cache["key"] != key:
        gidx_cat, sels, meta = prep3(nodes, neigh_idx)
        nc = build3(meta)
        _cache.update(key=key, nc=nc, meta=(gidx_cat, sels, meta))
    nc = _cache["nc"]
    gidx_cat, sels, meta = _cache["meta"]
    feat_f16 = features.astype(np.float16)
    in_maps = [
        {
            "features_f16": feat_f16,
            "gidx": np.ascontiguousarray(gidx_cat[c]),
            "sels": np.ascontiguousarray(
                sels[c].view(ml_dtypes.float8_e4m3)
            ),
        }
        for c in range(NCORES)
    ]
    res = bass_utils.run_bass_kernel_spmd(
        nc,
        in_maps,
        core_ids=list(range(NCORES)),
        trace=PROFILE,
        trace_cores=[0] if PROFILE else None,
    )
    if PROFILE:
        kernel.last_result = res
    out = np.concatenate(
        [res.results[c]["out"].T[:B_LOC] for c in range(NCORES)], axis=0
    )
    return np.ascontiguousarray(out, dtype=np.float32)
